# revision 13
# baseline (speedup 1.0000x reference)
"""Trainium2 Bass kernel for nn_FCNNShapeCounterValuationFunction.

Computes out[i] = 0.999 * a[i, int(z[i, 5])] for z:[B,32] f32, a:[B,16] f32.

Strategy (pure data parallel, 8 NeuronCores):
  - Shard rows across 8 cores (BC = B/8 rows each); per core view rows as
    [128 partitions, BC/128] with per-partition contiguous blocks so every
    DMA descriptor is a large contiguous chunk (strided per-row loads
    measured ~11ns/descriptor/engine — 3x slower than streaming).
  - HBM-bound: ~100 MB/core irreducible. Both HWDGE rings (SP + ACT)
    carry a balanced mix of z and a tiles (~50 MB each) so the queues
    drain together; measured steady rate ~415-425 GB/s/core.
  - Per round of up to 512 rows/partition: two z subtiles and two a
    half-tiles split across the rings. The software pipeline emits round
    r's loads, then round r-1's compute, then round r-2's store, so each
    engine's dma_starts sit AHEAD of any instruction that waits on data
    and load descriptor generation always runs a round ahead.
  - Compute (DVE): ACT extracts the index column; the 16-way gather runs
    as scalar_tensor_tensor ops prod[:,k,:] = (idx==k)*a[:,:,k] writing
    bf16; bf16 binary-tree add (2x DVE mode); x0.999 scale back to f32.
    bf16 adds <0.4% error vs the 2e-2 tolerance. The tail rounds shrink
    (256,128,128) to minimize the post-DMA serial compute chain.
  - GpSimd (SWDGE) issues the output stores (lagged one round).
"""

import numpy as np

B = 4194304
D = 32
K = 16
ATTR = 5
SCALE = 0.999
N_CORES = 8
P = 128
BC = B // N_CORES  # 524288 rows per core
F = 512  # rows per partition per round

_cache = {}


def _round_sizes(npp):
    """Rounds (rows/partition each): 512-row rounds with a shrinking tail
    to minimize the post-DMA compute tail."""
    assert npp % 512 == 0 and npp >= 512
    if npp == 512:
        return [256, 128, 128]
    return [512] * (npp // 512 - 1) + [256, 128, 128]


def _build(bc=BC, f=F):
    """Build + compile the per-core Bass program for bc rows."""
    from contextlib import ExitStack

    import concourse.tile as tile
    from concourse import bacc, mybir

    npp = bc // P  # rows per partition
    assert bc % P == 0
    rounds = _round_sizes(npp)

    nc = bacc.Bacc("TRN2", target_bir_lowering=False, debug=False, num_devices=N_CORES)
    z = nc.dram_tensor("z", [bc, D], mybir.dt.float32, kind="ExternalInput")
    a = nc.dram_tensor("a", [bc, K], mybir.dt.float32, kind="ExternalInput")
    out = nc.dram_tensor("out", [bc], mybir.dt.float32, kind="ExternalOutput")

    zv = z.ap().rearrange("(p n) d -> p n d", p=P)
    av = a.ap().rearrange("(p n) k -> p n k", p=P)
    ov = out.ap().rearrange("(p n) -> p n", p=P)

    f32 = mybir.dt.float32
    bf16 = mybir.dt.bfloat16
    eq = mybir.AluOpType.is_equal
    mult = mybir.AluOpType.mult
    add = mybir.AluOpType.add

    with ExitStack() as ctx:
        tc = ctx.enter_context(tile.TileContext(nc))
        zpool = ctx.enter_context(tc.tile_pool(name="zpool", bufs=2))
        apool = ctx.enter_context(tc.tile_pool(name="apool", bufs=3))
        ppool = ctx.enter_context(tc.tile_pool(name="ppool", bufs=2))
        spool = ctx.enter_context(tc.tile_pool(name="spool", bufs=2))

        # Software pipeline: iteration r emits loads(r), then the FULL
        # compute of round r-1 (extract -> one-hot -> tree -> scale), then
        # the store of round r-2. On ACT this orders the next round's
        # dma_starts BEFORE the extract (which waits on a z load), so
        # load descriptor generation always runs a round ahead and
        # z-buffer WAR never gates loads on compute backlog.
        prev = None  # (zt0, zt1, at, lo, hi, f) awaiting compute
        prev_store = None  # (sc, lo, hi) awaiting store

        def do_compute(state, store_state):
            zt0, zt1, at, clo, chi, cf = state
            idx = spool.tile([P, cf], f32, tag="idx", name="idx")
            nc.scalar.copy(idx[:, : cf // 2], zt0[:, :, ATTR])
            nc.scalar.copy(idx[:, cf // 2 :], zt1[:, :, ATTR])

            # One-hot gather on DVE (fused STT), bf16 out.
            prod = ppool.tile([P, K, cf], bf16, tag="prod", name="prod")
            for k in range(K):
                nc.vector.scalar_tensor_tensor(
                    prod[:, k, :], idx[:], float(k), at[:, :, k], eq, mult
                )
            # GpSimd: the round-before-last's output store (ready now).
            if store_state is not None:
                sc0, slo, shi = store_state
                nc.gpsimd.dma_start(ov[:, slo:shi], sc0[:])

            # DVE: bf16 tree-add (2x mode) + scale back to f32.
            for h in (8, 4, 2, 1):
                nc.vector.tensor_tensor(
                    prod[:, :h, :], prod[:, :h, :], prod[:, h : 2 * h, :], add
                )
            sc = spool.tile([P, cf], f32, tag="sc", name="sc")
            nc.vector.tensor_scalar_mul(sc[:], prod[:, 0, :], SCALE)
            return (sc, clo, chi)

        pos = 0
        for f in rounds:
            lo, hi = pos, pos + f
            mid = lo + f // 2
            pos = hi

            # Balanced rings: SP carries z-subtile-0 + a-half-1,
            # ACT carries a-half-0 + z-subtile-1 (~equal bytes each).
            zt0 = zpool.tile([P, f // 2, D], f32, tag="zt", name="zt")
            zt1 = zpool.tile([P, f - f // 2, D], f32, tag="zt", name="zt")
            at = apool.tile([P, f, K], f32, tag="at", name="at")
            nc.sync.dma_start(zt0[:], zv[:, lo:mid, :])
            nc.scalar.dma_start(at[:, : f // 2, :], av[:, lo:mid, :])
            nc.sync.dma_start(at[:, f // 2 :, :], av[:, mid:hi, :])
            nc.scalar.dma_start(zt1[:], zv[:, mid:hi, :])

            if prev is not None:
                prev_store = do_compute(prev, prev_store)
            prev = (zt0, zt1, at, lo, hi, f)

        prev_store = do_compute(prev, prev_store)
        sc0, slo, shi = prev_store
        nc.gpsimd.dma_start(ov[:, slo:shi], sc0[:])

    nc.compile()
    return nc


def _get(bc=BC, f=F):
    key = (bc, f)
    if key not in _cache:
        _cache[key] = _build(bc, f)
    return _cache[key]


def kernel(z, a, attr_index=5, **run_kwargs):
    """Full inputs in, full output out. Shards rows over 8 NeuronCores."""
    from concourse import bass_utils

    assert int(attr_index) == ATTR
    z = np.asarray(z, dtype=np.float32)
    a = np.asarray(a, dtype=np.float32)
    assert z.shape == (B, D) and a.shape == (B, K)

    nc = _get()
    in_maps = [
        {"z": z[c * BC : (c + 1) * BC], "a": a[c * BC : (c + 1) * BC]}
        for c in range(N_CORES)
    ]
    res = bass_utils.run_bass_kernel_spmd(
        nc, in_maps, core_ids=list(range(N_CORES)), **run_kwargs
    )
    out = np.concatenate([r["out"] for r in res.results], axis=0)
    if run_kwargs:
        kernel.last_results = res
    return out


# revision 14
# speedup vs baseline: 1.1988x; 1.1988x over previous
"""Trainium2 Bass kernel for nn_FCNNShapeCounterValuationFunction.

Computes out[i] = 0.999 * a[i, int(z[i, 5])] for z:[B,32] f32, a:[B,16] f32.

Strategy (pure data parallel, 8 NeuronCores):
  - Shard rows across 8 cores (BC = B/8 rows each); per core view rows as
    [128 partitions, BC/128] with per-partition contiguous blocks so every
    DMA descriptor is a large contiguous chunk (strided per-row loads
    measured ~11ns/descriptor/engine — 3x slower than streaming).
  - HBM-bound: ~100 MB/core irreducible. Both HWDGE rings (SP + ACT)
    carry a balanced mix of z and a tiles (~50 MB each) so the queues
    drain together; measured steady rate ~415-425 GB/s/core.
  - Per round of up to 512 rows/partition: two z subtiles and two a
    half-tiles split across the rings. The software pipeline emits round
    r's loads, then round r-1's compute, then round r-2's store, so each
    engine's dma_starts sit AHEAD of any instruction that waits on data
    and load descriptor generation always runs a round ahead.
  - Compute (DVE): ACT extracts the index column; the 16-way gather runs
    as scalar_tensor_tensor ops prod[:,k,:] = (idx==k)*a[:,:,k] writing
    bf16; bf16 binary-tree add (2x DVE mode); x0.999 scale back to f32.
    bf16 adds <0.4% error vs the 2e-2 tolerance. The tail rounds shrink
    (256,256,256,128,128) to minimize the post-DMA serial compute chain.
  - GpSimd (SWDGE) issues the output stores (lagged one round).
"""

import numpy as np

B = 4194304
D = 32
K = 16
ATTR = 5
SCALE = 0.999
N_CORES = 8
P = 128
BC = B // N_CORES  # 524288 rows per core
F = 512  # rows per partition per round

_cache = {}


def _round_sizes(npp):
    """Rounds (rows/partition each): 512-row rounds with a shrinking tail
    to minimize the post-DMA compute tail."""
    assert npp % 512 == 0 and npp >= 512
    if npp == 512:
        return [256, 128, 128]
    return [512] * (npp // 512 - 2) + [256, 256, 256, 128, 128]


def _build(bc=BC, f=F):
    """Build + compile the per-core Bass program for bc rows."""
    from contextlib import ExitStack

    import concourse.tile as tile
    from concourse import bacc, mybir

    npp = bc // P  # rows per partition
    assert bc % P == 0
    rounds = _round_sizes(npp)

    nc = bacc.Bacc("TRN2", target_bir_lowering=False, debug=False, num_devices=N_CORES)
    z = nc.dram_tensor("z", [bc, D], mybir.dt.float32, kind="ExternalInput")
    a = nc.dram_tensor("a", [bc, K], mybir.dt.float32, kind="ExternalInput")
    out = nc.dram_tensor("out", [bc], mybir.dt.float32, kind="ExternalOutput")

    zv = z.ap().rearrange("(p n) d -> p n d", p=P)
    av = a.ap().rearrange("(p n) k -> p n k", p=P)
    ov = out.ap().rearrange("(p n) -> p n", p=P)

    f32 = mybir.dt.float32
    bf16 = mybir.dt.bfloat16
    eq = mybir.AluOpType.is_equal
    mult = mybir.AluOpType.mult
    add = mybir.AluOpType.add

    with ExitStack() as ctx:
        tc = ctx.enter_context(tile.TileContext(nc))
        zpool = ctx.enter_context(tc.tile_pool(name="zpool", bufs=2))
        apool = ctx.enter_context(tc.tile_pool(name="apool", bufs=3))
        ppool = ctx.enter_context(tc.tile_pool(name="ppool", bufs=2))
        spool = ctx.enter_context(tc.tile_pool(name="spool", bufs=2))

        # Software pipeline: iteration r emits loads(r), then the FULL
        # compute of round r-1 (extract -> one-hot -> tree -> scale), then
        # the store of round r-2. On ACT this orders the next round's
        # dma_starts BEFORE the extract (which waits on a z load), so
        # load descriptor generation always runs a round ahead and
        # z-buffer WAR never gates loads on compute backlog.
        prev = None  # (zt0, zt1, at, lo, hi, f) awaiting compute
        prev_store = None  # (sc, lo, hi) awaiting store

        def do_compute(state, store_state):
            zt0, zt1, at, clo, chi, cf = state
            idx = spool.tile([P, cf], f32, tag="idx", name="idx")
            nc.scalar.copy(idx[:, : cf // 2], zt0[:, :, ATTR])
            nc.scalar.copy(idx[:, cf // 2 :], zt1[:, :, ATTR])

            # One-hot gather on DVE (fused STT), bf16 out.
            prod = ppool.tile([P, K, cf], bf16, tag="prod", name="prod")
            for k in range(K):
                nc.vector.scalar_tensor_tensor(
                    prod[:, k, :], idx[:], float(k), at[:, :, k], eq, mult
                )
            # GpSimd: the round-before-last's output store (ready now).
            if store_state is not None:
                sc0, slo, shi = store_state
                nc.gpsimd.dma_start(ov[:, slo:shi], sc0[:])

            # DVE: bf16 tree-add (2x mode) + scale back to f32.
            for h in (8, 4, 2, 1):
                nc.vector.tensor_tensor(
                    prod[:, :h, :], prod[:, :h, :], prod[:, h : 2 * h, :], add
                )
            sc = spool.tile([P, cf], f32, tag="sc", name="sc")
            nc.vector.tensor_scalar_mul(sc[:], prod[:, 0, :], SCALE)
            return (sc, clo, chi)

        pos = 0
        for f in rounds:
            lo, hi = pos, pos + f
            mid = lo + f // 2
            pos = hi

            # Balanced rings: SP carries z-subtile-0 + a-half-1,
            # ACT carries a-half-0 + z-subtile-1 (~equal bytes each).
            zt0 = zpool.tile([P, f // 2, D], f32, tag="zt", name="zt")
            zt1 = zpool.tile([P, f - f // 2, D], f32, tag="zt", name="zt")
            at = apool.tile([P, f, K], f32, tag="at", name="at")
            nc.sync.dma_start(zt0[:], zv[:, lo:mid, :])
            nc.scalar.dma_start(at[:, : f // 2, :], av[:, lo:mid, :])
            nc.sync.dma_start(at[:, f // 2 :, :], av[:, mid:hi, :])
            nc.scalar.dma_start(zt1[:], zv[:, mid:hi, :])

            if prev is not None:
                prev_store = do_compute(prev, prev_store)
            prev = (zt0, zt1, at, lo, hi, f)

        prev_store = do_compute(prev, prev_store)
        sc0, slo, shi = prev_store
        nc.gpsimd.dma_start(ov[:, slo:shi], sc0[:])

    nc.compile()
    return nc


def _get(bc=BC, f=F):
    key = (bc, f)
    if key not in _cache:
        _cache[key] = _build(bc, f)
    return _cache[key]


def kernel(z, a, attr_index=5, **run_kwargs):
    """Full inputs in, full output out. Shards rows over 8 NeuronCores."""
    from concourse import bass_utils

    assert int(attr_index) == ATTR
    z = np.asarray(z, dtype=np.float32)
    a = np.asarray(a, dtype=np.float32)
    assert z.shape == (B, D) and a.shape == (B, K)

    nc = _get()
    in_maps = [
        {"z": z[c * BC : (c + 1) * BC], "a": a[c * BC : (c + 1) * BC]}
        for c in range(N_CORES)
    ]
    res = bass_utils.run_bass_kernel_spmd(
        nc, in_maps, core_ids=list(range(N_CORES)), **run_kwargs
    )
    out = np.concatenate([r["out"] for r in res.results], axis=0)
    if run_kwargs:
        kernel.last_results = res
    return out
